# revision 88
# baseline (speedup 1.0000x reference)
"""Trainium2 Bass kernel for nn_DiffeqSolver (two-step Euler MLP-ODE).

Math (per trajectory n, time step i):
    f(y) = tanh(y@W1 + b1)@W2 + b2
    h_i = t_i / 2
    y1_i = y0 + h_i*f(y0)
    y2_i = y1_i + h_i*f(y1_i)
    out[n,i,:] = active[n,i] ? y2_i : 0      (active = any(mask[n,i,:] > 0))
    (t_i == 0 gives y2 == y0 exactly, so the reference's pos-branch is folded.)

Memory-bound: every DMA serializes on one DMA_ENGINES device at 360GB/s, so
DMA time ~= bytes moved. Per core: 16MB mask in (46.6us, irreducible) + the
output, staged as fp16 (8MB, 23.3us) and upcast to f32 on the host during
the unshard - compute is already bf16-staged (rel err ~3.4e-3 vs the 2e-2
gate; fp16 staging adds ~5e-4). Consts ~0.5MB: y0^T ships pre-transposed
bf16 straight into the YG-top/CY-bottom partitions (no PE transpose stage);
the per-step operands (W1S=[W1;hW1], h*W2, rhsD=[h I;I]) build on Pool.

With the half-rate output, the vector engines, not the DMA, set the pace:
  ACT  ~77us busy: 64x tanh [128,1024] (1038ns each - psZ is two PSUM banks
       so a bigger tanh cannot fit), the act table, stage-1 tanh, and ~13
       "ACT drains" (see below).
  DVE  ~72us busy: the mask reduce (16x int32 tensor_reduce over a
       [128,32,64] mask half, 2.26us each - tensor_reduce has no DVE fast
       mode, a 2-byte TT tree is no cheaper since TT only gets 2x, and the
       walrus backend rejects every integer op on Pool, so DVE owns all of
       it), ~51 psY drains (stt PSUM f32 -> masked fp16, ~658ns - the drain
       IS the masking pass), stage-1 biases, is_gt conversions.
  Pool ~51us: W1S/w2h/rhsD group builds (TT mult vs broadcast columns) and
       the fp16 0/1-mask multiply for ACT-drained blocks.
  PE   ~56us: z matmuls + psY accumulation (bf16 lhsT throughout).

Balance: blocks whose active bits would land after their natural drain slot,
plus enough low-margin blocks to offload ~13 of 64 drains from DVE, drain
via ACT (Identity, unmasked) with a deferred Pool fp16-multiply; the
endgame (group 7) alternates DVE/ACT per chunk so the last eight drains
overlap after the final tanh.

DMA stream (single SP queue; HWDGE ring holds ~4-10 outstanding 128-row
transfers, so service order == emission order): small critical const pack
(stage-1 weights) first, y0^T twice, the rest of the consts, then all 16
mask t-halves (half-0 of every chunk first - groups 0-3 only need half-0,
which keeps the late set small), then the output as 2-group 2KB-row packs
(728ns transfer > 650ns HWDGE config keeps the tail device-bound) with
single-group blocks for groups 6-7 so the drain-gated endgame ships
promptly. Reduce/phase-B/pack emissions are slotted into the main loop from
a calibrated timing model of mask arrivals and the tanh cadence.

Sharding: data-parallel over trajectories, 1024 per core x 8 cores.
"""

import numpy as np
from contextlib import ExitStack

import concourse.bass as bass
import concourse.bacc as bacc
import concourse.mybir as mybir
import concourse.tile as tile
from concourse.bass_utils import run_bass_kernel_spmd

N_TRAJ, N_TIME, LAT, HID, DIM = 8192, 64, 64, 128, 64
NCORES = 8
T = N_TRAJ // NCORES          # 1024 trajectories per core
NCH = T // 128                # 8 chunks of 128 trajectories
SG = 8                        # steps per group
NG = N_TIME // SG             # 8 step groups
NP = NG // 2                  # 4 output packs (2 groups each) per chunk
HT = N_TIME // 2              # 32 time steps per mask half
P = 128
F32 = mybir.dt.float32
F16 = mybir.dt.float16
BF16 = mybir.dt.bfloat16
I32 = mybir.dt.int32
I16 = mybir.dt.int16
AF = mybir.ActivationFunctionType
OP = mybir.AluOpType

_cache = {}
WPACKA_COLS = 291   # stage-1-critical + W1S-build inputs
WPACKB_COLS = 256   # w2h/rhsD-build inputs
DVE_FULL = set(range(NCH))      # every mask half reduces on DVE

# --- timing model (us) used to place reduce/phase-B/pack-DMA slots and the
# late line. Calibrated against the realized TimelineSim schedule: mask half
# transfers end 2.913us apart starting ~7.5us; main-loop tanh k ends
# ~(11.2 + 1.05k)us; block (g, c) drains ~0.6us after tanh 8g+8+c.
_TANH0 = 9.9
_STEP = 1.05


def _arr(c, h):
    return 8.4 + 2.913 * (8 * h + c) + 0.9


def _rdy(c, h):
    # Pool-assisted chunks lag when arrivals outpace the assist pipeline.
    return _arr(c, h) + 3.2


def _step_of(t_us):
    return max(0, int((t_us - _TANH0) / _STEP) + 1)


def _drain_step(g, c):
    return 8 * (g + 1) + c


def _margin(g, c):
    t_drain = _TANH0 + _STEP * _drain_step(g, c) + 0.6
    return t_drain - _rdy(c, g // 4)


def _act_drain(g, c):
    # smallest-margin blocks drain via ACT Identity (unmasked) + deferred
    # Pool fp16-mult; endgame odd chunks too (ACT is idle after tanh 63).
    if g == NG - 1 and c % 2 == 1:
        return True
    return _margin(g, c) < 2.5


def _dve_late(g, c):
    # mildly-late blocks: DVE unmasked copy + deferred Pool mult (no
    # activeH dependency, but keeps the work off the tanh-paced ACT).
    return not _act_drain(g, c) and _margin(g, c) < 5.0


def _deferred(g, c):
    return _act_drain(g, c) or _dve_late(g, c)


def _reduce_slots():
    slots = {}
    for h in range(2):
        for c in range(NCH):
            lag = 0.2
            k = min(_step_of(_arr(c, h) + lag), NG * SG - 1)
            slots.setdefault(divmod(k, SG), []).append((c, h))
    return slots


def _phase_b_step(gg, c):
    return max((gg + 2) * SG, _step_of(_rdy(c, gg // 4) + 0.4))


def _slot_b():
    slots = {}
    for gg in range(NG - 1):
        for c in range(NCH):
            if _deferred(gg, c):
                slots.setdefault(divmod(_phase_b_step(gg, c), SG), []).append((gg, c))
    return slots


def _pack_dma_slots():
    # pack p covers groups (2p, 2p+1) for p=0..2; groups 6 and 7 ship as
    # single-group DMAs (the endgame is drain-gated; small blocks leave
    # sooner). The pack DMA is emitted after both drains and any deferred
    # phase-B finalizers.
    slots = {}
    for p in range(NP - 1):
        for c in range(NCH):
            k = max(
                _drain_step(2 * p + 1, c) + 1,
                *[
                    _phase_b_step(gg, c) + 1
                    for gg in (2 * p, 2 * p + 1)
                    if _deferred(gg, c)
                ],
                *[0],
            )
            slots.setdefault(k, []).append((p, c))
    for c in range(NCH):
        slots.setdefault(_drain_step(6, c) + 1, []).append((6, c))
    return slots


SLOT_B = _slot_b()
REDUCE_SLOTS = _reduce_slots()
PACK_SLOTS = _pack_dma_slots()


def _emit(ctx, tc, nc, wpacka, wpackb, y0t, mask, out):
    const = ctx.enter_context(tc.tile_pool(name="const", bufs=1))

    wpa = const.tile([P, WPACKA_COLS], F32)
    nc.sync.dma_start(wpa[:], wpacka[:])
    W1x2b = wpa[:, 0:64].bitcast(BF16)     # [W1; W1] bf16
    W2b = wpa[:, 64:96].bitcast(BF16)
    b1_sb = wpa[:, 96:97]
    b2b = wpa[:, 97:98]                    # [b2; b2]
    b2x2 = wpa[:, 98:99]                   # rows 0:64 = 2*b2
    W1x2 = wpa[:, 99:227]                  # f32 [W1; W1]
    scale2 = wpa[:, 227:291]               # [1; h] per step (W1S build)

    # y0^T (bf16) straight into YG top and CY bottom partitions.
    YG = const.tile([P, T], BF16)
    CY = const.tile([P, T], BF16)
    nc.sync.dma_start(YG[0:LAT, :].bitcast(F32), y0t[:])
    nc.sync.dma_start(CY[LAT:P, :].bitcast(F32), y0t[:])

    wpb = const.tile([P, WPACKB_COLS], F32)
    nc.sync.dma_start(wpb[:], wpackb[:])
    W2_sb = wpb[:, 0:64]
    Hcol = wpb[:, 64:128]                  # h on all partitions (w2h)
    Ibf32 = wpb[:, 128:192]                # [I64; I64] f32
    Vflip = wpb[:, 192:256]                # [h; 1] per step (rhsD build)

    # ---- mask DMAs: t-halves, all half-0s then all half-1s, on SP before
    # any out DMA. Emission order == device service order.
    mp0 = ctx.enter_context(tc.tile_pool(name="mh0", bufs=5))
    mp1 = ctx.enter_context(tc.tile_pool(name="mh1", bufs=5))
    mts = {}
    for h in range(2):
        for c in range(NCH):
            mt = (mp0 if h == 0 else mp1).tile([P, HT * DIM], I32, tag="m")
            nc.sync.dma_start(
                mt[:], mask[c * P : (c + 1) * P, h * HT * DIM : (h + 1) * HT * DIM]
            )
            mts[(c, h)] = mt

    activeH = const.tile([P, NCH * N_TIME], F16)   # 0/1.0 active bits
    redp = ctx.enter_context(tc.tile_pool(name="red", bufs=2))

    def emit_reduce(c, h):
        red = redp.tile([P, HT], I32, tag="red")
        m = mts[(c, h)][:].rearrange("p (t d) -> p t d", d=DIM)
        nc.vector.tensor_reduce(
            red[:], m, axis=mybir.AxisListType.X, op=OP.max
        )
        sl = slice(c * N_TIME + h * HT, c * N_TIME + (h + 1) * HT)
        nc.vector.tensor_scalar(activeH[:, sl], red[:], 0, None, op0=OP.is_gt)

    ypsum = ctx.enter_context(tc.tile_pool(name="ypsum", bufs=4, space="PSUM"))
    zpsum = ctx.enter_context(tc.tile_pool(name="zpsum", bufs=2, space="PSUM"))

    # ---- stage 1, full-width (shortest startup chain): g0 = tanh(y0@W1+b1)
    # @W2; YG rows 64:128 = g0^T + b2, CY rows 0:64 = g0^T + 2*b2 (DVE).
    y0p = ctx.enter_context(tc.tile_pool(name="y0p", bufs=2))
    psA = zpsum.tile([P, T], F32, tag="psZ")
    for hlf in range(2):
        sl = slice(hlf * 512, (hlf + 1) * 512)
        nc.tensor.matmul(
            psA[:, sl], W1x2b[0:LAT, :], YG[0:LAT, sl],
            start=True, stop=True, skip_group_check=(hlf == 1),
        )
    u0 = y0p.tile([P, T], BF16, tag="u0")
    nc.scalar.activation(u0[:], psA[:], AF.Tanh, bias=b1_sb[:, 0:1])
    psG = zpsum.tile([P, T], F32, tag="psZ")
    for hlf in range(2):
        sl = slice(hlf * 512, (hlf + 1) * 512)
        nc.tensor.matmul(
            psG[LAT:P, sl], W2b[:], u0[:, sl],
            start=True, stop=True, skip_group_check=(hlf == 1),
        )
    for hlf in range(2):
        sl = slice(hlf * 512, (hlf + 1) * 512)
        nc.vector.tensor_scalar(
            YG[LAT:P, sl], psG[LAT:P, sl], b2b[LAT:P, 0:1], None, op0=OP.add
        )
    for hlf in range(2):
        sl = slice(hlf * 512, (hlf + 1) * 512)
        nc.tensor.matmul(
            psG[0:LAT, sl], W2b[:], u0[:, sl],
            start=True, stop=True, skip_group_check=True,
        )
    nc.vector.tensor_scalar(
        CY[0:LAT, :], psG[0:LAT, :], b2x2[0:LAT, 0:1], None, op0=OP.add
    )

    # ---- main loop.
    wpool = ctx.enter_context(tc.tile_pool(name="wpool", bufs=2))
    upool = ctx.enter_context(tc.tile_pool(name="upool", bufs=20))
    opool = ctx.enter_context(tc.tile_pool(name="opool", bufs=24))

    uts = [None] * (NG * SG)
    w2hs = [None] * NG
    rhds = [None] * NG
    packs = {}

    def emit_w_group(g):
        # stacked lhsT [W1; h W1], h*W2, rhsD [h_i I; I]: all built on Pool
        # as divides by host-shipped reciprocals ("divide" runs at 0.6 GPSIMD
        # efficiency vs 0.42 for multiply).
        gs = slice(g * SG, (g + 1) * SG)
        W1S = wpool.tile([P, SG * HID], BF16, tag="w1s", bufs=3)
        nc.gpsimd.tensor_mul(
            W1S[:].rearrange("p (s k) -> p s k", k=HID),
            W1x2[:][:, None, :].broadcast_to([P, SG, HID]),
            scale2[:, gs][:, :, None].broadcast_to([P, SG, HID]),
        )
        w2h = wpool.tile([HID, SG * LAT], BF16, tag="w2s", bufs=3)
        nc.gpsimd.tensor_mul(
            w2h[:].rearrange("p (s l) -> p s l", l=LAT),
            W2_sb[:][:, None, :].broadcast_to([HID, SG, LAT]),
            Hcol[:, gs][:, :, None].broadcast_to([HID, SG, LAT]),
        )
        rhd = wpool.tile([P, SG * LAT], BF16, tag="rhd", bufs=3)
        nc.gpsimd.tensor_mul(
            rhd[:].rearrange("p (s l) -> p s l", l=LAT),
            Ibf32[:][:, None, :].broadcast_to([P, SG, LAT]),
            Vflip[:, gs][:, :, None].broadcast_to([P, SG, LAT]),
        )
        return W1S, w2h, rhd

    def active_bc(gg, c, which=None):
        return activeH[:, c * N_TIME + gg * SG : c * N_TIME + (gg + 1) * SG][
            :, :, None
        ].broadcast_to([P, SG, LAT])

    def pack_half(gg, c):
        if gg >= 6:
            if (gg, c) not in packs:
                sg = opool.tile([P, SG * LAT], F16, tag="sg", bufs=16)
                packs[(gg, c)] = sg
            return packs[(gg, c)][:]
        p = gg // 2
        if (p, c) not in packs:
            pk = opool.tile([P, 2 * SG * LAT], F16, tag="pk", bufs=19)
            packs[(p, c)] = pk
        off = (gg % 2) * SG * LAT
        return packs[(p, c)][:, off : off + SG * LAT]

    def emit_phaseA(gg, c):
        """psY = y0 + h*Cb (CY@rhsD) + per-step bf16 u^T (h W2); drained into
        its pack half: masked stt if activeF is ready, plain copy otherwise
        (phase B Pool-ANDs the mask in later)."""
        csl = slice(c * P, (c + 1) * P)
        w2h = w2hs[gg]
        psY = ypsum.tile([P, SG * LAT], F32, tag="psY")
        nc.tensor.matmul(
            psY[:], CY[:, csl], rhds[gg][:],
            start=True, stop=False,
        )
        for s in range(SG):
            # the s==7 matmul carries the group stop and must NOT skip the
            # group check (skip_group_check bypasses the stop tracking).
            nc.tensor.matmul(
                psY[:, s * LAT : (s + 1) * LAT],
                uts[gg * SG + s][:, csl],
                w2h[:, s * LAT : (s + 1) * LAT],
                start=False,
                stop=(s == SG - 1),
                skip_group_check=(s != SG - 1),
            )
        dst = pack_half(gg, c)
        if gg == NG - 1 and c % 2 == 1:
            # endgame: ACT copies (idle after the last tanh), Pool masks.
            nc.scalar.activation(dst, psY[:], AF.Identity)
            nc.gpsimd.tensor_mul(
                dst.rearrange("p (s l) -> p s l", l=LAT),
                dst.rearrange("p (s l) -> p s l", l=LAT),
                active_bc(gg, c),
            )
        elif _act_drain(gg, c):
            nc.scalar.activation(dst, psY[:], AF.Identity)
        elif _dve_late(gg, c):
            nc.vector.tensor_copy(dst, psY[:])
        else:
            nc.vector.scalar_tensor_tensor(
                dst.rearrange("p (s l) -> p s l", l=LAT),
                psY[:].rearrange("p (s l) -> p s l", l=LAT),
                1.0,
                active_bc(gg, c),
                op0=OP.bypass,
                op1=OP.mult,
            )

    def emit_phaseB(gg, c):
        # zero inactive steps: Pool fp16 multiply by the 0/1 mask in place.
        dst = pack_half(gg, c)
        nc.gpsimd.tensor_mul(
            dst.rearrange("p (s l) -> p s l", l=LAT),
            dst.rearrange("p (s l) -> p s l", l=LAT),
            active_bc(gg, c),
        )

    def emit_pack_dma(p, c):
        if p >= 6:
            nc.sync.dma_start(
                out[c * P : (c + 1) * P, p * SG * LAT : (p + 1) * SG * LAT],
                packs.pop((p, c))[:],
            )
        else:
            nc.sync.dma_start(
                out[
                    c * P : (c + 1) * P,
                    2 * p * SG * LAT : 2 * (p + 1) * SG * LAT,
                ],
                packs.pop((p, c))[:],
            )

    W1S_all = [None] * NG
    for g in (0, 1):
        W1S_all[g], w2hs[g], rhds[g] = emit_w_group(g)

    for g in range(NG):
        for s in range(SG):
            k = g * SG + s
            psZ = zpsum.tile([P, 2 * 512], F32, tag="psZ")
            for hlf in range(2):
                sl = slice(hlf * 512, (hlf + 1) * 512)
                nc.tensor.matmul(
                    psZ[:, sl],
                    W1S_all[g][:, s * HID : (s + 1) * HID],
                    YG[:, sl],
                    start=True, stop=True, skip_group_check=True,
                )
            ut = upool.tile([P, T], BF16, tag="ut")
            nc.scalar.activation(ut[:], psZ[:], AF.Tanh, bias=b1_sb[:, 0:1])
            uts[k] = ut

            if s == 0 and 1 <= g and g + 1 < NG:
                W1S_all[g + 1], w2hs[g + 1], rhds[g + 1] = emit_w_group(g + 1)
            if g >= 1:
                emit_phaseA(g - 1, s)
            for ch in REDUCE_SLOTS.get((g, s), []):
                emit_reduce(*ch)
            for b in SLOT_B.get((g, s), []):
                emit_phaseB(*b)
            for pc in PACK_SLOTS.get(k, []):
                emit_pack_dma(*pc)
    for key in sorted(k for k in SLOT_B if k[0] >= NG):
        for b in SLOT_B[key]:
            emit_phaseB(*b)
    for k in sorted(k for k in PACK_SLOTS if k >= NG * SG):
        for pc in PACK_SLOTS[k]:
            emit_pack_dma(*pc)
    for c in range(NCH):
        emit_phaseA(NG - 1, c)
        emit_pack_dma(NG - 1, c)


def _build():
    if "nc" in _cache:
        return _cache["nc"]
    nc = bacc.Bacc("TRN2", target_bir_lowering=False, debug=False)
    wpacka = nc.dram_tensor("wpacka", [P, WPACKA_COLS], F32, kind="ExternalInput")
    wpackb = nc.dram_tensor("wpackb", [P, WPACKB_COLS], F32, kind="ExternalInput")
    y0t = nc.dram_tensor("y0t", [LAT, T // 2], F32, kind="ExternalInput")
    mask = nc.dram_tensor("mask", [T, N_TIME * DIM], I32, kind="ExternalInput")
    out = nc.dram_tensor("out", [T, N_TIME * LAT], F16, kind="ExternalOutput")

    with tile.TileContext(nc) as tc:
        with ExitStack() as ctx:
            _emit(ctx, tc, nc, wpacka, wpackb, y0t, mask, out)
    nc.compile()
    _cache["nc"] = nc
    return nc


def kernel(first_point, time_steps, mask, W1, b1, W2, b2, trace=False, **trace_kw):
    import ml_dtypes

    first_point = np.asarray(first_point)
    time_steps = np.asarray(time_steps)
    mask = np.asarray(mask)
    W1a = np.ascontiguousarray(np.asarray(W1), dtype=np.float32)
    b1a = np.ascontiguousarray(np.asarray(b1), dtype=np.float32).reshape(HID)
    W2a = np.ascontiguousarray(np.asarray(W2), dtype=np.float32)
    b2a = np.ascontiguousarray(np.asarray(b2), dtype=np.float32).reshape(LAT)
    h = np.ascontiguousarray(time_steps, dtype=np.float32).reshape(N_TIME) * 0.5

    fp_full = np.ascontiguousarray(first_point[0], dtype=np.float32)  # [8192, 64]
    mask_full = np.ascontiguousarray(mask, dtype=np.int32).reshape(
        N_TRAJ, N_TIME * DIM
    )

    W1x2 = np.vstack([W1a, W1a])                            # [128, 128]
    eye2 = np.vstack([np.eye(LAT, dtype=np.float32)] * 2)   # [128, 64]
    pa = np.zeros((P, WPACKA_COLS), dtype=np.float32)
    pa[:, 0:64] = W1x2.astype(ml_dtypes.bfloat16).view(np.float32)
    pa[:, 64:96] = W2a.astype(ml_dtypes.bfloat16).view(np.float32)
    pa[:, 96] = b1a
    pa[:, 97] = np.concatenate([b2a, b2a])
    pa[0:LAT, 98] = 2.0 * b2a
    pa[:, 99:227] = W1x2
    pa[0:LAT, 227:291] = 1.0                                # scale2 top
    pa[LAT:P, 227:291] = np.tile(h, (LAT, 1))               # scale2 bottom

    pb = np.zeros((P, WPACKB_COLS), dtype=np.float32)
    pb[:, 0:64] = W2a
    pb[:, 64:128] = np.tile(h, (P, 1))                      # Hcol
    pb[:, 128:192] = eye2                                   # Ibf32
    pb[0:LAT, 192:256] = np.tile(h, (LAT, 1))               # Vflip top
    pb[LAT:P, 192:256] = 1.0                                # Vflip bottom

    nc = _build()
    in_maps = []
    for c in range(NCORES):
        sl = slice(c * T, (c + 1) * T)
        y0tc = np.ascontiguousarray(
            fp_full[sl].T.astype(ml_dtypes.bfloat16)  # [64 lat, 1024 traj]
        ).view(np.float32)                            # [64, 512] f32-view
        in_maps.append(
            {
                "wpacka": pa,
                "wpackb": pb,
                "y0t": y0tc,
                "mask": np.ascontiguousarray(mask_full[sl]),
            }
        )

    res = run_bass_kernel_spmd(
        nc, in_maps, core_ids=list(range(NCORES)), trace=trace, **trace_kw
    )
    outs = [
        r["out"].astype(np.float32).reshape(T, N_TIME, 1, LAT) for r in res.results
    ]
    full = np.concatenate(outs, axis=0)
    if trace:
        kernel.last_result = res
    return full


# revision 90
# speedup vs baseline: 1.0056x; 1.0056x over previous
"""Trainium2 Bass kernel for nn_DiffeqSolver (two-step Euler MLP-ODE).

Math (per trajectory n, time step i):
    f(y) = tanh(y@W1 + b1)@W2 + b2
    h_i = t_i / 2
    y1_i = y0 + h_i*f(y0)
    y2_i = y1_i + h_i*f(y1_i)
    out[n,i,:] = active[n,i] ? y2_i : 0      (active = any(mask[n,i,:] > 0))
    (t_i == 0 gives y2 == y0 exactly, so the reference's pos-branch is folded.)

Memory-bound: every DMA serializes on one DMA_ENGINES device at 360GB/s, so
DMA time ~= bytes moved. Per core: 16MB mask in (46.6us, irreducible) + the
output, staged as fp16 (8MB, 23.3us) and upcast to f32 on the host during
the unshard - compute is already bf16-staged (rel err ~3.4e-3 vs the 2e-2
gate; fp16 staging adds ~5e-4). Consts ~0.5MB: y0^T ships pre-transposed
bf16 straight into the YG-top/CY-bottom partitions (no PE transpose stage);
the per-step operands (W1S=[W1;hW1], h*W2, rhsD=[h I;I]) build on Pool.

With the half-rate output, the vector engines, not the DMA, set the pace:
  ACT  ~77us busy: 64x tanh [128,1024] (1038ns each - psZ is two PSUM banks
       so a bigger tanh cannot fit), the act table, stage-1 tanh, and ~13
       "ACT drains" (see below).
  DVE  ~72us busy: the mask reduce (16x int32 tensor_reduce over a
       [128,32,64] mask half, 2.26us each - tensor_reduce has no DVE fast
       mode, a 2-byte TT tree is no cheaper since TT only gets 2x, and the
       walrus backend rejects every integer op on Pool, so DVE owns all of
       it), ~51 psY drains (stt PSUM f32 -> masked fp16, ~658ns - the drain
       IS the masking pass), stage-1 biases, is_gt conversions.
  Pool ~51us: W1S/w2h/rhsD group builds (TT mult vs broadcast columns) and
       the fp16 0/1-mask multiply for ACT-drained blocks.
  PE   ~56us: z matmuls + psY accumulation (bf16 lhsT throughout).

Balance: blocks whose active bits would land after their natural drain slot,
plus enough low-margin blocks to offload ~13 of 64 drains from DVE, drain
via ACT (Identity, unmasked) with a deferred Pool fp16-multiply; the
endgame (group 7) alternates DVE/ACT per chunk so the last eight drains
overlap after the final tanh.

DMA stream (single SP queue; HWDGE ring holds ~4-10 outstanding 128-row
transfers, so service order == emission order): small critical const pack
(stage-1 weights) first, y0^T twice, the rest of the consts, then all 16
mask t-halves (half-0 of every chunk first - groups 0-3 only need half-0,
which keeps the late set small), then the output as 2-group 2KB-row packs
(728ns transfer > 650ns HWDGE config keeps the tail device-bound) with
single-group blocks for groups 6-7 so the drain-gated endgame ships
promptly. Reduce/phase-B/pack emissions are slotted into the main loop from
a calibrated timing model of mask arrivals and the tanh cadence.

Sharding: data-parallel over trajectories, 1024 per core x 8 cores.
"""

import numpy as np
from contextlib import ExitStack

import concourse.bass as bass
import concourse.bacc as bacc
import concourse.mybir as mybir
import concourse.tile as tile
from concourse.bass_utils import run_bass_kernel_spmd

N_TRAJ, N_TIME, LAT, HID, DIM = 8192, 64, 64, 128, 64
NCORES = 8
T = N_TRAJ // NCORES          # 1024 trajectories per core
NCH = T // 128                # 8 chunks of 128 trajectories
SG = 8                        # steps per group
NG = N_TIME // SG             # 8 step groups
NP = NG // 2                  # 4 output packs (2 groups each) per chunk
HT = N_TIME // 2              # 32 time steps per mask half
P = 128
F32 = mybir.dt.float32
F16 = mybir.dt.float16
BF16 = mybir.dt.bfloat16
I32 = mybir.dt.int32
I16 = mybir.dt.int16
AF = mybir.ActivationFunctionType
OP = mybir.AluOpType

_cache = {}
WPACKA_COLS = 291   # stage-1-critical + W1S-build inputs
WPACKB_COLS = 256   # w2h/rhsD-build inputs
DVE_FULL = set(range(NCH))      # every mask half reduces on DVE

# --- timing model (us) used to place reduce/phase-B/pack-DMA slots and the
# late line. Calibrated against the realized TimelineSim schedule: mask half
# transfers end 2.913us apart starting ~7.5us; main-loop tanh k ends
# ~(11.2 + 1.05k)us; block (g, c) drains ~0.6us after tanh 8g+8+c.
_TANH0 = 9.9
_STEP = 1.05


def _arr(c, h):
    return 8.4 + 2.913 * (8 * h + c) + 0.9


def _rdy(c, h):
    # Pool-assisted chunks lag when arrivals outpace the assist pipeline.
    return _arr(c, h) + 3.2


def _step_of(t_us):
    return max(0, int((t_us - _TANH0) / _STEP) + 1)


def _drain_step(g, c):
    return 8 * (g + 1) + c


def _margin(g, c):
    t_drain = _TANH0 + _STEP * _drain_step(g, c) + 0.6
    return t_drain - _rdy(c, g // 4)


def _act_drain(g, c):
    # smallest-margin blocks drain via ACT Identity (unmasked) + deferred
    # Pool fp16-mult; endgame odd chunks too (ACT is idle after tanh 63).
    if g == NG - 1:
        return False
    return _margin(g, c) < 2.5


def _dve_late(g, c):
    # mildly-late blocks: DVE unmasked copy + deferred Pool mult (no
    # activeH dependency, but keeps the work off the tanh-paced ACT).
    return not _act_drain(g, c) and _margin(g, c) < 5.0


def _deferred(g, c):
    return _act_drain(g, c) or _dve_late(g, c)


def _reduce_slots():
    slots = {}
    for h in range(2):
        for c in range(NCH):
            lag = 0.2
            k = min(_step_of(_arr(c, h) + lag), NG * SG - 1)
            slots.setdefault(divmod(k, SG), []).append((c, h))
    return slots


def _phase_b_step(gg, c):
    return max((gg + 2) * SG, _step_of(_rdy(c, gg // 4) + 0.4))


def _slot_b():
    slots = {}
    for gg in range(NG - 1):
        for c in range(NCH):
            if _deferred(gg, c):
                slots.setdefault(divmod(_phase_b_step(gg, c), SG), []).append((gg, c))
    return slots


def _pack_dma_slots():
    # pack p covers groups (2p, 2p+1) for p=0..2; groups 6 and 7 ship as
    # single-group DMAs (the endgame is drain-gated; small blocks leave
    # sooner). The pack DMA is emitted after both drains and any deferred
    # phase-B finalizers.
    slots = {}
    for p in range(NP - 1):
        for c in range(NCH):
            k = max(
                _drain_step(2 * p + 1, c) + 1,
                *[
                    _phase_b_step(gg, c) + 1
                    for gg in (2 * p, 2 * p + 1)
                    if _deferred(gg, c)
                ],
                *[0],
            )
            slots.setdefault(k, []).append((p, c))
    for c in range(NCH):
        slots.setdefault(_drain_step(6, c) + 1, []).append((6, c))
    return slots


SLOT_B = _slot_b()
REDUCE_SLOTS = _reduce_slots()
PACK_SLOTS = _pack_dma_slots()


def _emit(ctx, tc, nc, wpacka, wpackb, y0t, mask, out):
    const = ctx.enter_context(tc.tile_pool(name="const", bufs=1))

    wpa = const.tile([P, WPACKA_COLS], F32)
    nc.sync.dma_start(wpa[:], wpacka[:])
    W1x2b = wpa[:, 0:64].bitcast(BF16)     # [W1; W1] bf16
    W2b = wpa[:, 64:96].bitcast(BF16)
    b1_sb = wpa[:, 96:97]
    b2b = wpa[:, 97:98]                    # [b2; b2]
    b2x2 = wpa[:, 98:99]                   # rows 0:64 = 2*b2
    W1x2 = wpa[:, 99:227]                  # f32 [W1; W1]
    scale2 = wpa[:, 227:291]               # [1; h] per step (W1S build)

    # y0^T (bf16) straight into YG top and CY bottom partitions.
    YG = const.tile([P, T], BF16)
    CY = const.tile([P, T], BF16)
    nc.sync.dma_start(YG[0:LAT, :].bitcast(F32), y0t[:])
    nc.sync.dma_start(CY[LAT:P, :].bitcast(F32), y0t[:])

    wpb = const.tile([P, WPACKB_COLS], F32)
    nc.sync.dma_start(wpb[:], wpackb[:])
    W2_sb = wpb[:, 0:64]
    Hcol = wpb[:, 64:128]                  # h on all partitions (w2h)
    Ibf32 = wpb[:, 128:192]                # [I64; I64] f32
    Vflip = wpb[:, 192:256]                # [h; 1] per step (rhsD build)

    # ---- mask DMAs: t-halves, all half-0s then all half-1s, on SP before
    # any out DMA. Emission order == device service order.
    mp0 = ctx.enter_context(tc.tile_pool(name="mh0", bufs=5))
    mp1 = ctx.enter_context(tc.tile_pool(name="mh1", bufs=5))
    mts = {}
    for h in range(2):
        for c in range(NCH):
            mt = (mp0 if h == 0 else mp1).tile([P, HT * DIM], I32, tag="m")
            nc.sync.dma_start(
                mt[:], mask[c * P : (c + 1) * P, h * HT * DIM : (h + 1) * HT * DIM]
            )
            mts[(c, h)] = mt

    activeH = const.tile([P, NCH * N_TIME], F16)   # 0/1.0 active bits
    redp = ctx.enter_context(tc.tile_pool(name="red", bufs=2))

    def emit_reduce(c, h):
        red = redp.tile([P, HT], I32, tag="red")
        m = mts[(c, h)][:].rearrange("p (t d) -> p t d", d=DIM)
        nc.vector.tensor_reduce(
            red[:], m, axis=mybir.AxisListType.X, op=OP.max
        )
        sl = slice(c * N_TIME + h * HT, c * N_TIME + (h + 1) * HT)
        nc.vector.tensor_scalar(activeH[:, sl], red[:], 0, None, op0=OP.is_gt)

    ypsum = ctx.enter_context(tc.tile_pool(name="ypsum", bufs=4, space="PSUM"))
    zpsum = ctx.enter_context(tc.tile_pool(name="zpsum", bufs=2, space="PSUM"))

    # ---- stage 1, full-width (shortest startup chain): g0 = tanh(y0@W1+b1)
    # @W2; YG rows 64:128 = g0^T + b2, CY rows 0:64 = g0^T + 2*b2 (DVE).
    y0p = ctx.enter_context(tc.tile_pool(name="y0p", bufs=2))
    psA = zpsum.tile([P, T], F32, tag="psZ")
    for hlf in range(2):
        sl = slice(hlf * 512, (hlf + 1) * 512)
        nc.tensor.matmul(
            psA[:, sl], W1x2b[0:LAT, :], YG[0:LAT, sl],
            start=True, stop=True, skip_group_check=(hlf == 1),
        )
    u0 = y0p.tile([P, T], BF16, tag="u0")
    nc.scalar.activation(u0[:], psA[:], AF.Tanh, bias=b1_sb[:, 0:1])
    psG = zpsum.tile([P, T], F32, tag="psZ")
    for hlf in range(2):
        sl = slice(hlf * 512, (hlf + 1) * 512)
        nc.tensor.matmul(
            psG[LAT:P, sl], W2b[:], u0[:, sl],
            start=True, stop=True, skip_group_check=(hlf == 1),
        )
    for hlf in range(2):
        sl = slice(hlf * 512, (hlf + 1) * 512)
        nc.vector.tensor_scalar(
            YG[LAT:P, sl], psG[LAT:P, sl], b2b[LAT:P, 0:1], None, op0=OP.add
        )
    for hlf in range(2):
        sl = slice(hlf * 512, (hlf + 1) * 512)
        nc.tensor.matmul(
            psG[0:LAT, sl], W2b[:], u0[:, sl],
            start=True, stop=True, skip_group_check=True,
        )
    nc.vector.tensor_scalar(
        CY[0:LAT, :], psG[0:LAT, :], b2x2[0:LAT, 0:1], None, op0=OP.add
    )

    # ---- main loop.
    wpool = ctx.enter_context(tc.tile_pool(name="wpool", bufs=2))
    upool = ctx.enter_context(tc.tile_pool(name="upool", bufs=20))
    opool = ctx.enter_context(tc.tile_pool(name="opool", bufs=24))

    uts = [None] * (NG * SG)
    w2hs = [None] * NG
    rhds = [None] * NG
    packs = {}

    def emit_w_group(g):
        # stacked lhsT [W1; h W1], h*W2, rhsD [h_i I; I]: all built on Pool
        # as divides by host-shipped reciprocals ("divide" runs at 0.6 GPSIMD
        # efficiency vs 0.42 for multiply).
        gs = slice(g * SG, (g + 1) * SG)
        W1S = wpool.tile([P, SG * HID], BF16, tag="w1s", bufs=3)
        nc.gpsimd.tensor_mul(
            W1S[:].rearrange("p (s k) -> p s k", k=HID),
            W1x2[:][:, None, :].broadcast_to([P, SG, HID]),
            scale2[:, gs][:, :, None].broadcast_to([P, SG, HID]),
        )
        w2h = wpool.tile([HID, SG * LAT], BF16, tag="w2s", bufs=3)
        nc.gpsimd.tensor_mul(
            w2h[:].rearrange("p (s l) -> p s l", l=LAT),
            W2_sb[:][:, None, :].broadcast_to([HID, SG, LAT]),
            Hcol[:, gs][:, :, None].broadcast_to([HID, SG, LAT]),
        )
        rhd = wpool.tile([P, SG * LAT], BF16, tag="rhd", bufs=3)
        nc.gpsimd.tensor_mul(
            rhd[:].rearrange("p (s l) -> p s l", l=LAT),
            Ibf32[:][:, None, :].broadcast_to([P, SG, LAT]),
            Vflip[:, gs][:, :, None].broadcast_to([P, SG, LAT]),
        )
        return W1S, w2h, rhd

    def active_bc(gg, c, which=None):
        return activeH[:, c * N_TIME + gg * SG : c * N_TIME + (gg + 1) * SG][
            :, :, None
        ].broadcast_to([P, SG, LAT])

    def pack_half(gg, c):
        if gg >= 6:
            if (gg, c) not in packs:
                sg = opool.tile([P, SG * LAT], F16, tag="sg", bufs=16)
                packs[(gg, c)] = sg
            return packs[(gg, c)][:]
        p = gg // 2
        if (p, c) not in packs:
            pk = opool.tile([P, 2 * SG * LAT], F16, tag="pk", bufs=19)
            packs[(p, c)] = pk
        off = (gg % 2) * SG * LAT
        return packs[(p, c)][:, off : off + SG * LAT]

    def emit_phaseA(gg, c):
        """psY = y0 + h*Cb (CY@rhsD) + per-step bf16 u^T (h W2); drained into
        its pack half: masked stt if activeF is ready, plain copy otherwise
        (phase B Pool-ANDs the mask in later)."""
        csl = slice(c * P, (c + 1) * P)
        w2h = w2hs[gg]
        psY = ypsum.tile([P, SG * LAT], F32, tag="psY")
        nc.tensor.matmul(
            psY[:], CY[:, csl], rhds[gg][:],
            start=True, stop=False,
        )
        for s in range(SG):
            # the s==7 matmul carries the group stop and must NOT skip the
            # group check (skip_group_check bypasses the stop tracking).
            nc.tensor.matmul(
                psY[:, s * LAT : (s + 1) * LAT],
                uts[gg * SG + s][:, csl],
                w2h[:, s * LAT : (s + 1) * LAT],
                start=False,
                stop=(s == SG - 1),
                skip_group_check=(s != SG - 1),
            )
        dst = pack_half(gg, c)
        if _act_drain(gg, c):
            nc.scalar.activation(dst, psY[:], AF.Identity)
        elif _dve_late(gg, c):
            nc.vector.tensor_copy(dst, psY[:])
        else:
            nc.vector.scalar_tensor_tensor(
                dst.rearrange("p (s l) -> p s l", l=LAT),
                psY[:].rearrange("p (s l) -> p s l", l=LAT),
                1.0,
                active_bc(gg, c),
                op0=OP.bypass,
                op1=OP.mult,
            )

    def emit_phaseB(gg, c):
        # zero inactive steps: Pool fp16 multiply by the 0/1 mask in place.
        dst = pack_half(gg, c)
        nc.gpsimd.tensor_mul(
            dst.rearrange("p (s l) -> p s l", l=LAT),
            dst.rearrange("p (s l) -> p s l", l=LAT),
            active_bc(gg, c),
        )

    def emit_pack_dma(p, c):
        if p >= 6:
            nc.sync.dma_start(
                out[c * P : (c + 1) * P, p * SG * LAT : (p + 1) * SG * LAT],
                packs.pop((p, c))[:],
            )
        else:
            nc.sync.dma_start(
                out[
                    c * P : (c + 1) * P,
                    2 * p * SG * LAT : 2 * (p + 1) * SG * LAT,
                ],
                packs.pop((p, c))[:],
            )

    W1S_all = [None] * NG
    for g in (0, 1):
        W1S_all[g], w2hs[g], rhds[g] = emit_w_group(g)

    for g in range(NG):
        for s in range(SG):
            k = g * SG + s
            psZ = zpsum.tile([P, 2 * 512], F32, tag="psZ")
            for hlf in range(2):
                sl = slice(hlf * 512, (hlf + 1) * 512)
                nc.tensor.matmul(
                    psZ[:, sl],
                    W1S_all[g][:, s * HID : (s + 1) * HID],
                    YG[:, sl],
                    start=True, stop=True, skip_group_check=True,
                )
            ut = upool.tile([P, T], BF16, tag="ut")
            nc.scalar.activation(ut[:], psZ[:], AF.Tanh, bias=b1_sb[:, 0:1])
            uts[k] = ut

            if s == 0 and 1 <= g and g + 1 < NG:
                W1S_all[g + 1], w2hs[g + 1], rhds[g + 1] = emit_w_group(g + 1)
            if g >= 1:
                emit_phaseA(g - 1, s)
            for ch in REDUCE_SLOTS.get((g, s), []):
                emit_reduce(*ch)
            for b in SLOT_B.get((g, s), []):
                emit_phaseB(*b)
            for pc in PACK_SLOTS.get(k, []):
                emit_pack_dma(*pc)
    for key in sorted(k for k in SLOT_B if k[0] >= NG):
        for b in SLOT_B[key]:
            emit_phaseB(*b)
    for k in sorted(k for k in PACK_SLOTS if k >= NG * SG):
        for pc in PACK_SLOTS[k]:
            emit_pack_dma(*pc)
    for c in range(NCH):
        emit_phaseA(NG - 1, c)
        emit_pack_dma(NG - 1, c)


def _build():
    if "nc" in _cache:
        return _cache["nc"]
    nc = bacc.Bacc("TRN2", target_bir_lowering=False, debug=False)
    wpacka = nc.dram_tensor("wpacka", [P, WPACKA_COLS], F32, kind="ExternalInput")
    wpackb = nc.dram_tensor("wpackb", [P, WPACKB_COLS], F32, kind="ExternalInput")
    y0t = nc.dram_tensor("y0t", [LAT, T // 2], F32, kind="ExternalInput")
    mask = nc.dram_tensor("mask", [T, N_TIME * DIM], I32, kind="ExternalInput")
    out = nc.dram_tensor("out", [T, N_TIME * LAT], F16, kind="ExternalOutput")

    with tile.TileContext(nc) as tc:
        with ExitStack() as ctx:
            _emit(ctx, tc, nc, wpacka, wpackb, y0t, mask, out)
    nc.compile()
    _cache["nc"] = nc
    return nc


def kernel(first_point, time_steps, mask, W1, b1, W2, b2, trace=False, **trace_kw):
    import ml_dtypes

    first_point = np.asarray(first_point)
    time_steps = np.asarray(time_steps)
    mask = np.asarray(mask)
    W1a = np.ascontiguousarray(np.asarray(W1), dtype=np.float32)
    b1a = np.ascontiguousarray(np.asarray(b1), dtype=np.float32).reshape(HID)
    W2a = np.ascontiguousarray(np.asarray(W2), dtype=np.float32)
    b2a = np.ascontiguousarray(np.asarray(b2), dtype=np.float32).reshape(LAT)
    h = np.ascontiguousarray(time_steps, dtype=np.float32).reshape(N_TIME) * 0.5

    fp_full = np.ascontiguousarray(first_point[0], dtype=np.float32)  # [8192, 64]
    mask_full = np.ascontiguousarray(mask, dtype=np.int32).reshape(
        N_TRAJ, N_TIME * DIM
    )

    W1x2 = np.vstack([W1a, W1a])                            # [128, 128]
    eye2 = np.vstack([np.eye(LAT, dtype=np.float32)] * 2)   # [128, 64]
    pa = np.zeros((P, WPACKA_COLS), dtype=np.float32)
    pa[:, 0:64] = W1x2.astype(ml_dtypes.bfloat16).view(np.float32)
    pa[:, 64:96] = W2a.astype(ml_dtypes.bfloat16).view(np.float32)
    pa[:, 96] = b1a
    pa[:, 97] = np.concatenate([b2a, b2a])
    pa[0:LAT, 98] = 2.0 * b2a
    pa[:, 99:227] = W1x2
    pa[0:LAT, 227:291] = 1.0                                # scale2 top
    pa[LAT:P, 227:291] = np.tile(h, (LAT, 1))               # scale2 bottom

    pb = np.zeros((P, WPACKB_COLS), dtype=np.float32)
    pb[:, 0:64] = W2a
    pb[:, 64:128] = np.tile(h, (P, 1))                      # Hcol
    pb[:, 128:192] = eye2                                   # Ibf32
    pb[0:LAT, 192:256] = np.tile(h, (LAT, 1))               # Vflip top
    pb[LAT:P, 192:256] = 1.0                                # Vflip bottom

    nc = _build()
    in_maps = []
    for c in range(NCORES):
        sl = slice(c * T, (c + 1) * T)
        y0tc = np.ascontiguousarray(
            fp_full[sl].T.astype(ml_dtypes.bfloat16)  # [64 lat, 1024 traj]
        ).view(np.float32)                            # [64, 512] f32-view
        in_maps.append(
            {
                "wpacka": pa,
                "wpackb": pb,
                "y0t": y0tc,
                "mask": np.ascontiguousarray(mask_full[sl]),
            }
        )

    res = run_bass_kernel_spmd(
        nc, in_maps, core_ids=list(range(NCORES)), trace=trace, **trace_kw
    )
    outs = [
        r["out"].astype(np.float32).reshape(T, N_TIME, 1, LAT) for r in res.results
    ]
    full = np.concatenate(outs, axis=0)
    if trace:
        kernel.last_result = res
    return full
